# revision 8
# baseline (speedup 1.0000x reference)
"""Trainium2 Bass kernel for nn_EulerMisorientation3D (v2).

reference math (per voxel, Bunge ZXZ Euler angles scaled by [2pi, pi, 2pi]):
    g    = euler_to_matrix(x * scale)       (3x3 rotation)
    g_h  = euler_to_matrix(x_hat * scale)
    tr   = sum_i g_h[i,i] * inv(g)[i,i]
    out  = mean( arccos(0.5*(tr-1))^2 )

Per-voxel closed form (alpha=2pi*x0, beta=pi*x1, gamma=2pi*x2):
    u = cos(2pi*s), v = cos(2pi*t) with s = x0+x2, t = x0-x2
    c = cos(pi*x1)
    P4 = 4*(1+z) = (S+2)*(1+c*ch) + D*(c+ch)
        with U2 = u*uh, V2 = v*vh, S = U2+V2, D = U2-V2, z = 0.5*(tr-1)
    theta = arccos(z) = pi/2 + 2*atan(2*sigmoid(0.5*(ln Q4 - ln P4)) - 1)
        (Q4 = 8-P4; tanh(w) = 2*sigmoid(2w)-1 folded into atan's scale/bias;
         sigmoid is used instead of tanh so that sigmoid/arctan/square all
         live in the same ACT table set -> one fewer ACT_TABLE_LOAD.)

Engine split:
  GPSIMD: scalar_tensor_tensor (0.25 + x0) +- x2 -> fp16 (the +0.25 is the
    wrap shift, pre-added so the DVE wrap is a single-scalar-op mod).
  DVE: mm = m mod 1 via tensor_scalar fp16->fp16 (hits the 4x_2p DVE perf
    mode: 2-byte packed SBUF operands), then the product chain in fp16
    (plain tensor_tensor ops run in 2x mode; only one scalar_tensor_tensor).
  ACT: Sin(2pi*mm - pi) = -cos(2pi*s) (sign cancels in pair products),
    Sin(pi*x1 - pi/2) = -cos(beta), then Ln x2 / Sigmoid / Arctan /
    Square(+accum).  Table chain: trig_and_small -> natural_log ->
    sigmoid_and_others = 3 loads.

Sharding: flattened voxel axis split over 8 cores; each core reduces its
262144 voxels to [P, 2] partial sums; host sums (fp64) and divides by N.
"""

import math

import numpy as np

import concourse.bacc as bacc
import concourse.tile as tile
from concourse.tile_rust import add_dep_helper
from concourse import mybir
from concourse.bass_utils import run_bass_kernel_spmd

F32 = mybir.dt.float32
F16 = mybir.dt.float16
AF = mybir.ActivationFunctionType
OP = mybir.AluOpType

N_CORES = 8
NVOX = 128 * 128 * 128          # 2097152 voxels
PER = NVOX // N_CORES           # 262144 voxels per core
P = 128                         # SBUF partitions
COLS = PER // P                 # 2048 free-dim columns per core
T = 4                           # front tiles
FD = COLS // T                  # columns per tile
NH = 2                          # tail halves
HD = COLS // NH                 # columns per half (1024)
TPH = T // NH                   # tiles per half

PI = math.pi
LN_EPS = 2e-4


def build_bass():
    nc = bacc.Bacc("TRN2", target_bir_lowering=False, debug=False,
                   num_devices=N_CORES)
    xs = nc.declare_dram_parameter("xs", [3, PER], F32, isOutput=False)
    xh = nc.declare_dram_parameter("xh", [3, PER], F32, isOutput=False)
    out = nc.declare_dram_parameter("o", [P, NH], F32, isOutput=True)

    xs_v = xs[:].rearrange("c (p q) -> p c q", p=P)
    xh_v = xh[:].rearrange("c (p q) -> p c q", p=P)

    with tile.TileContext(nc) as tc:
        with (
            tc.tile_pool(name="io", bufs=T) as io,
            tc.tile_pool(name="wk", bufs=3) as wk,
            tc.tile_pool(name="half", bufs=NH) as hp,
            tc.tile_pool(name="big", bufs=1) as big,
        ):
            acc = big.tile([P, NH], F32, tag="acc")
            x1b = big.tile([P, 2, COLS], F32, tag="x1b")
            sb = big.tile([P, 2, COLS], F16, tag="sb")

            b_mpi2 = big.tile([P, 1], F32, tag="b_mpi2")
            b_eps = big.tile([P, 1], F32, tag="b_eps")
            b_eps8 = big.tile([P, 1], F32, tag="b_eps8")
            b_m1 = big.tile([P, 1], F32, tag="b_m1")
            b_ppi2 = big.tile([P, 1], F32, tag="b_ppi2")
            nc.vector.memset(b_mpi2, -PI / 2)
            nc.vector.memset(b_eps, LN_EPS)
            nc.vector.memset(b_eps8, 8.0 + LN_EPS)
            nc.vector.memset(b_m1, -1.0)
            nc.vector.memset(b_ppi2, PI / 2)

            # ---- DMAs: the xs stream rides the SP HWDGE ring, the xh
            # stream the ACT ring (issued before ACT's first sin, i.e.
            # during the window where ACT waits on gpsimd anyway); x1 bulk
            # loads ride mid-stream (only needed by the sb sins, which run
            # after each half's su4 sins).
            in02s = []
            for j in range(T):
                in02s.append(io.tile([P, 2, 2, FD], F32, tag="in02",
                                     name=f"in02_{j}"))

            def pair_dma(j):
                sl = slice(j * FD, (j + 1) * FD)
                nc.sync.dma_start(out=in02s[j][:, 0, :, :],
                                  in_=xs_v[:, 0:3:2, sl])
                nc.scalar.dma_start(out=in02s[j][:, 1, :, :],
                                 in_=xh_v[:, 0:3:2, sl])

            pair_dma(0)
            pair_dma(1)
            nc.sync.dma_start(out=x1b[:, 0, :], in_=xs_v[:, 1, :])
            nc.scalar.dma_start(out=x1b[:, 1, :], in_=xh_v[:, 1, :])
            pair_dma(2)
            pair_dma(3)

            act_chain = []   # ACT instrs in required queue order
            mods = []        # per-tile DVE mod instr (for ordering hints)
            svds = []        # per-tile (uv2, S, D) early products
            pqs = [None] * NH   # per-half P4 buffers
            sigs = [None] * NH  # per-half (sig, pi3p)
            dds = []

            # ---- phase 1 per tile: gpsimd pre-add, DVE mod, ACT sin ----
            for j in range(T):
                in02 = in02s[j]
                m4 = wk.tile([P, 4, FD], F32, tag="m4")
                # m4 rows: s_x|s_h (=x0+x2), t_x|t_h (=x0-x2); the wrap
                # shifts (-0.75 for s in [0,2), +0.25 for t in (-1,1)) land
                # both in [-0.5, 0.5] with a single +-1 correction, and
                # sin(2pi*m) = cos(2pi*(x0+-x2)) with no bias/sign flip.
                nc.gpsimd.tensor_add(m4[:, 0:2, :], in02[:, :, 0, :],
                                     in02[:, :, 1, :])
                nc.gpsimd.tensor_sub(m4[:, 2:4, :], in02[:, :, 0, :],
                                     in02[:, :, 1, :])
                nc.vector.add_range_wrap(
                    m4[:, 0:2, :], m4[:, 0:2, :], -0.75, 0.5, 1.0)
                mod = nc.vector.add_range_wrap(
                    m4[:, 2:4, :], m4[:, 2:4, :], 0.25, 0.5, 1.0)
                mods.append(mod)
                # su4 = sin(2pi*m) = (u_x, u_h, v_x, v_h)
                su4 = wk.tile([P, 4, FD], F16, tag="su4")
                act_chain.append(nc.scalar.activation(
                    su4[:], m4[:], AF.Sin, bias=0.0, scale=2 * PI))

                # early products: U2|V2, S, D
                uv2 = wk.tile([P, 2, FD], F16, tag="uv2")
                svd = wk.tile([P, 2, FD], F16, tag="svd")
                i_uv = nc.vector.tensor_mul(
                    uv2[:], su4[:, 0::2, :], su4[:, 1::2, :])
                nc.vector.tensor_add(svd[:, 0, :], uv2[:, 0, :], uv2[:, 1, :])
                i_d = nc.vector.tensor_sub(
                    svd[:, 1, :], uv2[:, 0, :], uv2[:, 1, :])
                svds.append((svd, i_uv, i_d))

                # sb sins for each half ride after that half's su4 sins
                if j % TPH == TPH - 1:
                    h = j // TPH
                    hs = slice(h * HD, (h + 1) * HD)
                    act_chain.append(nc.scalar.activation(
                        sb[:, 0, hs], x1b[:, 0, hs], AF.Sin,
                        bias=b_mpi2[:], scale=PI))
                    act_chain.append(nc.scalar.activation(
                        sb[:, 1, hs], x1b[:, 1, hs], AF.Sin,
                        bias=b_mpi2[:], scale=PI))

            # keep tile j's early products behind tile j+1's mod on DVE
            for j in range(T - 1):
                svd, i_uv, i_d = svds[j]
                add_dep_helper(i_uv.ins, mods[j + 1].ins, sync=False,
                               reason="products behind next mod")

            # ---- per-half bulk: sig = sb+sbh (= -(c+ch)), pi3p = 1+c*ch;
            # then per-tile late products A/B/P4 ----
            for h in range(NH):
                hs = slice(h * HD, (h + 1) * HD)
                sg = hp.tile([P, 2, HD], F16, tag="sg")
                nc.vector.tensor_add(sg[:, 0, :], sb[:, 0, hs], sb[:, 1, hs])
                nc.vector.tensor_mul(sg[:, 1, :], sb[:, 0, hs], sb[:, 1, hs])
                nc.vector.tensor_scalar(
                    sg[:, 1, :], sg[:, 1, :], 1.0, None, OP.add)
                sigs[h] = sg
                pq = hp.tile([P, TPH, FD], F16, tag="pq")
                pqs[h] = pq
                for k in range(TPH):
                    j = h * TPH + k
                    svd = svds[j][0]
                    ks = slice(k * FD, (k + 1) * FD)
                    ab = wk.tile([P, 2, FD], F16, tag="ab")
                    # A = (2+S) * pi3p
                    nc.vector.scalar_tensor_tensor(
                        ab[:, 0, :], svd[:, 0, :], 2.0, sg[:, 1, ks],
                        OP.add, OP.mult)
                    # B = D * sig  (sig = -(c+ch), so P4 = A - B)
                    nc.vector.tensor_mul(
                        ab[:, 1, :], svd[:, 1, :], sg[:, 0, ks])
                    nc.vector.tensor_sub(
                        pq[:, k, :], ab[:, 0, :], ab[:, 1, :])

            # ---- tail per half: ln x2 -> dd -> sigmoid -> atan -> sq ----
            lns = []
            for h in range(NH):
                ln = hp.tile([P, 2, TPH, FD], F16, tag="ln")
                act_chain.append(nc.scalar.activation(
                    ln[:, 0, :, :], pqs[h][:], AF.Ln,
                    bias=b_eps[:], scale=1.0))
                act_chain.append(nc.scalar.activation(
                    ln[:, 1, :, :], pqs[h][:], AF.Ln,
                    bias=b_eps8[:], scale=-1.0))
                lns.append(ln)
            for h in range(NH):
                dd = hp.tile([P, TPH, FD], F16, tag="dd")
                nc.vector.tensor_sub(
                    dd[:], lns[h][:, 1, :, :], lns[h][:, 0, :, :])
                dds.append(dd)
            # tanh(0.25*dd) = 2*sigmoid(0.5*dd) - 1; the 2x-1 is folded
            # into Arctan's scale/bias.  theta = pi/2 + 2*atan(...).
            for h in range(NH):
                act_chain.append(nc.scalar.activation(
                    dds[h][:], dds[h][:], AF.Sigmoid, bias=0.0, scale=0.5))
            for h in range(NH):
                act_chain.append(nc.scalar.activation(
                    dds[h][:], dds[h][:], AF.Arctan, bias=b_m1[:], scale=2.0))
            for h in range(NH):
                act_chain.append(nc.scalar.activation(
                    dds[h][:], dds[h][:], AF.Square,
                    bias=b_ppi2[:], scale=2.0,
                    accum_out=acc[:, h:h + 1]))

            # Pin the ACT queue order so each spline table set loads once:
            # sin* (trig_and_small) -> ln* (natural_log) ->
            # sigmoid*/atan*/square* (sigmoid_and_others).
            for a, b in zip(act_chain, act_chain[1:]):
                add_dep_helper(b.ins, a.ins, sync=False,
                               reason="ACT table-set ordering")

            nc.sync.dma_start(out=out[:], in_=acc[:])

    nc.compile()
    return nc


_CACHE = {}


def _get_nc():
    if "nc" not in _CACHE:
        _CACHE["nc"] = build_bass()
    return _CACHE["nc"]


def _run(x, x_hat, **spmd_kwargs):
    x = np.ascontiguousarray(np.asarray(x, dtype=np.float32).reshape(3, NVOX))
    xh = np.ascontiguousarray(np.asarray(x_hat, dtype=np.float32).reshape(3, NVOX))

    in_maps = []
    for c in range(N_CORES):
        sl = slice(c * PER, (c + 1) * PER)
        in_maps.append({
            "xs": np.ascontiguousarray(x[:, sl]),
            "xh": np.ascontiguousarray(xh[:, sl]),
        })

    nc = _get_nc()
    res = run_bass_kernel_spmd(
        nc, in_maps, core_ids=list(range(N_CORES)), **spmd_kwargs)
    total = 0.0
    for r in res.results:
        total += r["o"].astype(np.float64).sum()
    return np.float32(total / NVOX), res


def kernel(x: np.ndarray, x_hat: np.ndarray) -> np.ndarray:
    val, _ = _run(x, x_hat)
    return val


# revision 11
# speedup vs baseline: 1.0760x; 1.0760x over previous
"""Trainium2 Bass kernel for nn_EulerMisorientation3D (v2).

reference math (per voxel, Bunge ZXZ Euler angles scaled by [2pi, pi, 2pi]):
    g    = euler_to_matrix(x * scale)       (3x3 rotation)
    g_h  = euler_to_matrix(x_hat * scale)
    tr   = sum_i g_h[i,i] * inv(g)[i,i]
    out  = mean( arccos(0.5*(tr-1))^2 )

Per-voxel closed form (alpha=2pi*x0, beta=pi*x1, gamma=2pi*x2):
    u = cos(2pi*s), v = cos(2pi*t) with s = x0+x2, t = x0-x2
    c = cos(pi*x1)
    P4 = 4*(1+z) = (S+2)*(1+c*ch) + D*(c+ch)
        with U2 = u*uh, V2 = v*vh, S = U2+V2, D = U2-V2, z = 0.5*(tr-1)
    theta = arccos(z) = pi/2 + 2*atan(2*sigmoid(0.5*(ln Q4 - ln P4)) - 1)
        (Q4 = 8-P4; tanh(w) = 2*sigmoid(2w)-1 folded into atan's scale/bias;
         sigmoid is used instead of tanh so that sigmoid/arctan/square all
         live in the same ACT table set -> one fewer ACT_TABLE_LOAD.)

Engine split:
  GPSIMD: scalar_tensor_tensor (0.25 + x0) +- x2 -> fp16 (the +0.25 is the
    wrap shift, pre-added so the DVE wrap is a single-scalar-op mod).
  DVE: mm = m mod 1 via tensor_scalar fp16->fp16 (hits the 4x_2p DVE perf
    mode: 2-byte packed SBUF operands), then the product chain in fp16
    (plain tensor_tensor ops run in 2x mode; only one scalar_tensor_tensor).
  ACT: Sin(2pi*mm - pi) = -cos(2pi*s) (sign cancels in pair products),
    Sin(pi*x1 - pi/2) = -cos(beta), then Ln x2 / Sigmoid / Arctan /
    Square(+accum).  Table chain: trig_and_small -> natural_log ->
    sigmoid_and_others = 3 loads.

Sharding: flattened voxel axis split over 8 cores; each core reduces its
262144 voxels to [P, 2] partial sums; host sums (fp64) and divides by N.
"""

import math

import numpy as np

import concourse.bacc as bacc
import concourse.tile as tile
from concourse.tile_rust import add_dep_helper
from concourse import mybir
from concourse.bass_utils import run_bass_kernel_spmd

F32 = mybir.dt.float32
F16 = mybir.dt.float16
AF = mybir.ActivationFunctionType
OP = mybir.AluOpType

N_CORES = 8
NVOX = 128 * 128 * 128          # 2097152 voxels
PER = NVOX // N_CORES           # 262144 voxels per core
P = 128                         # SBUF partitions
COLS = PER // P                 # 2048 free-dim columns per core
T = 4                           # front tiles
FD = COLS // T                  # columns per tile
NH = 2                          # tail halves
HD = COLS // NH                 # columns per half (1024)
TPH = T // NH                   # tiles per half

PI = math.pi
LN_EPS = 2e-4


def build_bass():
    nc = bacc.Bacc("TRN2", target_bir_lowering=False, debug=False,
                   num_devices=N_CORES)
    xs = nc.declare_dram_parameter("xs", [3, PER], F32, isOutput=False)
    xh = nc.declare_dram_parameter("xh", [3, PER], F32, isOutput=False)
    out = nc.declare_dram_parameter("o", [P, NH], F32, isOutput=True)

    xs_v = xs[:].rearrange("c (p q) -> p c q", p=P)
    xh_v = xh[:].rearrange("c (p q) -> p c q", p=P)

    with tile.TileContext(nc) as tc:
        with (
            tc.tile_pool(name="io", bufs=1) as io,
            tc.tile_pool(name="wk", bufs=3) as wk,
            tc.tile_pool(name="half", bufs=NH) as hp,
            tc.tile_pool(name="big", bufs=1) as big,
        ):
            acc = big.tile([P, NH], F32, tag="acc")
            x1b = big.tile([P, 2, COLS], F32, tag="x1b")
            sb = big.tile([P, 2, COLS], F16, tag="sb")

            b_mpi2 = big.tile([P, 1], F32, tag="b_mpi2")
            b_eps = big.tile([P, 1], F32, tag="b_eps")
            b_eps8 = big.tile([P, 1], F32, tag="b_eps8")
            b_m1 = big.tile([P, 1], F32, tag="b_m1")
            b_ppi2 = big.tile([P, 1], F32, tag="b_ppi2")
            nc.vector.memset(b_mpi2, -PI / 2)
            nc.vector.memset(b_eps, LN_EPS)
            nc.vector.memset(b_eps8, 8.0 + LN_EPS)
            nc.vector.memset(b_m1, -1.0)
            nc.vector.memset(b_ppi2, PI / 2)

            # ---- DMAs: the xs stream rides the SP HWDGE ring, the xh
            # stream the ACT ring (issued before ACT's first sin, i.e.
            # during the window where ACT waits on gpsimd anyway); x1 bulk
            # loads ride mid-stream (only needed by the sb sins, which run
            # after each half's su4 sins).
            in02s = []
            for j in range(T):
                in02s.append(io.tile([P, 2, 2, FD], F32, tag=f"in02_{j}",
                                     name=f"in02_{j}"))

            def pair_dma(j):
                sl = slice(j * FD, (j + 1) * FD)
                nc.sync.dma_start(out=in02s[j][:, 0, :, :],
                                  in_=xs_v[:, 0:3:2, sl])
                nc.scalar.dma_start(out=in02s[j][:, 1, :, :],
                                 in_=xh_v[:, 0:3:2, sl])

            for j in range(T):
                pair_dma(j)
            # x1 bulks last: only the sb sins need them, mid sin-phase
            nc.sync.dma_start(out=x1b[:, 0, :], in_=xs_v[:, 1, :])
            nc.scalar.dma_start(out=x1b[:, 1, :], in_=xh_v[:, 1, :])

            act_chain = []   # ACT instrs in required queue order
            mods = []        # per-tile DVE mod instr (for ordering hints)
            svds = []        # per-tile (uv2, S, D) early products
            pqs = [None] * NH   # per-half P4 buffers
            sigs = [None] * NH  # per-half (sig, pi3p)
            dds = []

            # ---- phase 1 per tile: gpsimd pre-add, DVE mod, ACT sin ----
            for j in range(T):
                in02 = in02s[j]
                m4 = wk.tile([P, 4, FD], F32, tag="m4")
                # m4 rows: s_x|s_h (=x0+x2) via gpsimd; -t_x|-t_h (=x2-x0)
                # via DVE STT (2*x2 - s); cos is even so -t == t for v.
                # Wrap shifts (-0.75 for s in [0,2), +0.25 for -t in (-1,1))
                # land both in [-0.5, 0.5] with a single +-1 correction, and
                # sin(2pi*m) = cos(2pi*(x0+-x2)) with no bias/sign flip.
                nc.gpsimd.tensor_add(m4[:, 0:2, :], in02[:, :, 0, :],
                                     in02[:, :, 1, :])
                nc.vector.scalar_tensor_tensor(
                    m4[:, 2:4, :], in02[:, :, 1, :], 2.0, m4[:, 0:2, :],
                    OP.mult, OP.subtract)
                nc.vector.add_range_wrap(
                    m4[:, 0:2, :], m4[:, 0:2, :], -0.75, 0.5, 1.0)
                mod = nc.vector.add_range_wrap(
                    m4[:, 2:4, :], m4[:, 2:4, :], 0.25, 0.5, 1.0)
                mods.append(mod)
                # su4 = sin(2pi*m) = (u_x, u_h, v_x, v_h)
                su4 = wk.tile([P, 4, FD], F16, tag="su4")
                act_chain.append(nc.scalar.activation(
                    su4[:], m4[:], AF.Sin, bias=0.0, scale=2 * PI))

                # early products: U2|V2, S, D
                uv2 = wk.tile([P, 2, FD], F16, tag="uv2")
                svd = wk.tile([P, 2, FD], F16, tag="svd")
                i_uv = nc.vector.tensor_mul(
                    uv2[:], su4[:, 0::2, :], su4[:, 1::2, :])
                nc.vector.tensor_add(svd[:, 0, :], uv2[:, 0, :], uv2[:, 1, :])
                i_d = nc.vector.tensor_sub(
                    svd[:, 1, :], uv2[:, 0, :], uv2[:, 1, :])
                svds.append((svd, i_uv, i_d))

                # sb sins for each half ride after that half's su4 sins
                if j % TPH == TPH - 1:
                    h = j // TPH
                    hs = slice(h * HD, (h + 1) * HD)
                    act_chain.append(nc.scalar.activation(
                        sb[:, 0, hs], x1b[:, 0, hs], AF.Sin,
                        bias=b_mpi2[:], scale=PI))
                    act_chain.append(nc.scalar.activation(
                        sb[:, 1, hs], x1b[:, 1, hs], AF.Sin,
                        bias=b_mpi2[:], scale=PI))

            # keep tile j's early products behind tile j+1's mod on DVE
            for j in range(T - 1):
                svd, i_uv, i_d = svds[j]
                add_dep_helper(i_uv.ins, mods[j + 1].ins, sync=False,
                               reason="products behind next mod")

            # ---- per-half bulk: sig = sb+sbh (= -(c+ch)), pi3p = 1+c*ch;
            # then per-tile late products A/B/P4 ----
            for h in range(NH):
                hs = slice(h * HD, (h + 1) * HD)
                sg = hp.tile([P, 2, HD], F16, tag="sg")
                nc.vector.tensor_add(sg[:, 0, :], sb[:, 0, hs], sb[:, 1, hs])
                nc.vector.tensor_mul(sg[:, 1, :], sb[:, 0, hs], sb[:, 1, hs])
                nc.vector.tensor_scalar(
                    sg[:, 1, :], sg[:, 1, :], 1.0, None, OP.add)
                sigs[h] = sg
                pq = hp.tile([P, TPH, FD], F16, tag="pq")
                pqs[h] = pq
                for k in range(TPH):
                    j = h * TPH + k
                    svd = svds[j][0]
                    ks = slice(k * FD, (k + 1) * FD)
                    ab = wk.tile([P, 2, FD], F16, tag="ab")
                    # A = (2+S) * pi3p
                    nc.vector.scalar_tensor_tensor(
                        ab[:, 0, :], svd[:, 0, :], 2.0, sg[:, 1, ks],
                        OP.add, OP.mult)
                    # B = D * sig  (sig = -(c+ch), so P4 = A - B)
                    nc.vector.tensor_mul(
                        ab[:, 1, :], svd[:, 1, :], sg[:, 0, ks])
                    nc.vector.tensor_sub(
                        pq[:, k, :], ab[:, 0, :], ab[:, 1, :])

            # ---- tail per half: ln x2 -> dd -> sigmoid -> atan -> sq ----
            lns = []
            for h in range(NH):
                ln = hp.tile([P, 2, TPH, FD], F16, tag="ln")
                act_chain.append(nc.scalar.activation(
                    ln[:, 0, :, :], pqs[h][:], AF.Ln,
                    bias=b_eps[:], scale=1.0))
                act_chain.append(nc.scalar.activation(
                    ln[:, 1, :, :], pqs[h][:], AF.Ln,
                    bias=b_eps8[:], scale=-1.0))
                lns.append(ln)
            for h in range(NH):
                dd = hp.tile([P, TPH, FD], F16, tag="dd")
                nc.vector.tensor_sub(
                    dd[:], lns[h][:, 1, :, :], lns[h][:, 0, :, :])
                dds.append(dd)
            # tanh(0.25*dd) = 2*sigmoid(0.5*dd) - 1; the 2x-1 is folded
            # into Arctan's scale/bias.  theta = pi/2 + 2*atan(...).
            for h in range(NH):
                act_chain.append(nc.scalar.activation(
                    dds[h][:], dds[h][:], AF.Sigmoid, bias=0.0, scale=0.5))
            for h in range(NH):
                act_chain.append(nc.scalar.activation(
                    dds[h][:], dds[h][:], AF.Arctan, bias=b_m1[:], scale=2.0))
            for h in range(NH):
                act_chain.append(nc.scalar.activation(
                    dds[h][:], dds[h][:], AF.Square,
                    bias=b_ppi2[:], scale=2.0,
                    accum_out=acc[:, h:h + 1]))

            # Pin the ACT queue order so each spline table set loads once:
            # sin* (trig_and_small) -> ln* (natural_log) ->
            # sigmoid*/atan*/square* (sigmoid_and_others).
            for a, b in zip(act_chain, act_chain[1:]):
                add_dep_helper(b.ins, a.ins, sync=False,
                               reason="ACT table-set ordering")

            nc.sync.dma_start(out=out[:], in_=acc[:])

    nc.compile()
    return nc


_CACHE = {}


def _get_nc():
    if "nc" not in _CACHE:
        _CACHE["nc"] = build_bass()
    return _CACHE["nc"]


def _run(x, x_hat, **spmd_kwargs):
    x = np.ascontiguousarray(np.asarray(x, dtype=np.float32).reshape(3, NVOX))
    xh = np.ascontiguousarray(np.asarray(x_hat, dtype=np.float32).reshape(3, NVOX))

    in_maps = []
    for c in range(N_CORES):
        sl = slice(c * PER, (c + 1) * PER)
        in_maps.append({
            "xs": np.ascontiguousarray(x[:, sl]),
            "xh": np.ascontiguousarray(xh[:, sl]),
        })

    nc = _get_nc()
    res = run_bass_kernel_spmd(
        nc, in_maps, core_ids=list(range(N_CORES)), **spmd_kwargs)
    total = 0.0
    for r in res.results:
        total += r["o"].astype(np.float64).sum()
    return np.float32(total / NVOX), res


def kernel(x: np.ndarray, x_hat: np.ndarray) -> np.ndarray:
    val, _ = _run(x, x_hat)
    return val
